# revision 16
# baseline (speedup 1.0000x reference)
"""AutoDeepFM forward on 8 Trainium2 NeuronCores (Bass/Tile), v3.

Key structure (data-parallel over batch, 64 rows/core):
  - Embedding table + all weights are baked into the NEFF as Const DRAM
    tensors (inline_tensor): loaded to HBM once at model-load, NOT bound
    per exec. Per-exec inputs are just the ids and the fp32 feature matrix
    (~50 KB), which collapses the per-exec dispatch overhead.
  - Gathers are chunked indirect DMAs ([128,1] offsets -- the only SWDGE
    ucode-supported shape), with flat orders chosen so the gather lands
    DIRECTLY in the compute layouts (no DRAM bounce):
      geo [128, 32, 32]: order n = b*64+f  -> even-b rows at partitions
        0..38, odd-b at 64..102, i.e. the field-major FM layout split in
        two partition groups (which then run matmuls CONCURRENTLY in
        disjoint PE row groups).
      hbx [128, 20, 16]: f-pair chunks -> batch-major MLP input split in
        two partition groups (f-even / f-odd), fixed up by PE transposes
        with a host-side permuted W1.
  - Batch order inside the kernel is even/odd interleaved; the host
    un-permutes after gathering results.
  - MLP in mixed fp8(weights, x16) x bf16(activations); scale undone via
    activation scale. 2nd/3rd order FM as in v2 but per b-parity half.
"""

import os
import functools
from itertools import combinations

import numpy as np
import ml_dtypes

import concourse.bass as bass
import concourse.mybir as mybir
import concourse.tile as tile
from concourse import bacc
from concourse.bass_utils import run_bass_kernel_spmd

BF16 = ml_dtypes.bfloat16
FP8 = ml_dtypes.float8_e4m3

B, F, E, V = 512, 39, 16, 1_000_000
N_CORES = 8
BC = B // N_CORES  # 64 batch rows per core
H = 700
PP = 768  # padded pair count (6 x 128)
KH = 6  # K chunks for hidden layers (700 -> 768)
MT = 6  # M tiles for hidden dims (700 -> 5x128+60)
BN_EPS = 1e-5
WS = 16.0  # fp8 weight scale

NGEO = 32  # geo gather calls (b-pair chunks)
NHB = 20   # hbx gather calls (f-pair chunks)

# W1 K-space: even-f block (20 f-slots x 16 = 320 rows, 3 chunks of
# 128/128/64) then odd-f block (20 slots x 16 = 320 rows incl. dummy f=39).
K1C = [128, 128, 64, 128, 128, 64]  # per-chunk K sizes, chunks 0-2 E, 3-5 O

# wq (fp8) column offsets: w1 6x700, w2 6x700, w3 6x700, w4 6
OFF_W1 = 0
OFF_W2 = 6 * H
OFF_W3 = 12 * H
OFF_W4 = 18 * H
CQ = 18 * H + KH

# wb (bf16) columns; sell/w3m/aupt live at partitions 0:39 AND 64:103
OFF_SEL = 0            # sell [39, 768] (both bases)
OFF_W3M = 768          # w3m [39, 768] (both bases)
OFF_SELR = 2 * 768     # selr [128, 6, 39]
OFF_AUP = OFF_SELR + 6 * F  # aupt [39, 39] (both bases)
OFF_ID = OFF_AUP + F   # ident [64, 64] (both bases)
CB = OFF_ID + 64

# fsml (fp32) columns: b1 [0:6], b2 [6:12], b3 [12:18], ones 18, wlin 19
CF = 20

PAIRS_JG = [(i, j) for j in range(1, F) for i in range(j)]

# even/odd batch permutation: kernel column j holds batch row BEO[j]
BEO = np.array([2 * j for j in range(BC // 2)] +
               [2 * j + 1 for j in range(BC // 2)], np.int64)


def _m_size(mc):
    return 128 if mc < MT - 1 else H - 128 * (MT - 1)  # 60 for the last tile


def _build(consts, cnst_f):
    """consts: dict name -> np.ndarray baked into the NEFF."""
    stage = os.environ.get("KSTAGE", "full")
    do_gather = stage in ("gather", "mlp", "fm2", "fm3", "full")
    do_mlp = stage in ("mlp", "fm2", "fm3", "full")
    do_fm2 = stage in ("fm2", "fm3", "full")
    do_fm3 = stage in ("fm3", "full")
    nc = bacc.Bacc("TRN2", target_bir_lowering=False, debug=False,
                   num_devices=N_CORES)
    dt = mybir.dt

    evps = nc.inline_tensor(consts["evps"], name="evps")
    wq = nc.inline_tensor(consts["wq"], name="wq")
    wb = nc.inline_tensor(consts["wb"], name="wb")
    fsml = nc.inline_tensor(consts["fsml"], name="fsml")

    # single merged per-exec input: cols 0:52 int32 gather offsets,
    # cols 52:116 fp32 xT bit-packed (one buffer = one PJRT arg; each extra
    # arg costs ~230us/exec in this dispatch path)
    inp = nc.dram_tensor("inp", [128, NGEO + NHB + BC], dt.int32,
                         kind="ExternalInput")
    out_d = nc.dram_tensor("out", [1, BC], dt.float32, kind="ExternalOutput")

    relu = mybir.ActivationFunctionType.Relu
    copyf = mybir.ActivationFunctionType.Copy

    with tile.TileContext(nc) as tc:
        with (
            tc.tile_pool(name="cst", bufs=1) as cst,
            tc.tile_pool(name="stream", bufs=2) as strm,
            tc.tile_pool(name="ps_small", bufs=2, space="PSUM") as psS,
            tc.tile_pool(name="ps_hr", bufs=1, space="PSUM") as psHR,
            tc.tile_pool(name="ps_lg", bufs=4, space="PSUM") as psLG,
        ):
            # ---- per-exec input loads ----
            NIX = NGEO + NHB
            idx_sb = cst.tile([128, NIX], dt.int32)
            nc.sync.dma_start(out=idx_sb[:], in_=inp.ap()[:, 0:NIX])
            xt_sb = cst.tile([F, BC], dt.float32)
            nc.sync.dma_start(
                out=xt_sb[:],
                in_=inp.ap()[0:F, NIX:NIX + BC].bitcast(dt.float32))

            # ---- chunked embedding gathers (one offset per partition) ----
            geo = cst.tile([128, NGEO, 2 * E], dt.bfloat16)
            hbx = cst.tile([128, NHB, E], dt.bfloat16)
            if do_gather:
                for c in range(NGEO):
                    nc.gpsimd.indirect_dma_start(
                        out=geo[:, c, :], out_offset=None, in_=evps.ap(),
                        in_offset=bass.IndirectOffsetOnAxis(
                            ap=idx_sb[:, c:c + 1], axis=0))
                for k in range(NHB):
                    nc.gpsimd.indirect_dma_start(
                        out=hbx[:, k, :], out_offset=None, in_=evps.ap(),
                        in_offset=bass.IndirectOffsetOnAxis(
                            ap=idx_sb[:, NGEO + k:NGEO + k + 1], axis=0))

            # ---- constant loads (overlap with gathers) ----
            wq_sb = cst.tile([128, CQ], dt.float8e4)
            if do_mlp:
                nc.sync.dma_start(out=wq_sb[:], in_=wq.ap())
            wb_sb = cst.tile([128, CB], dt.bfloat16)
            if do_mlp or do_fm2 or do_fm3:
                nc.sync.dma_start(out=wb_sb[:], in_=wb.ap())
            fs_sb = cst.tile([128, CF], dt.float32)
            nc.sync.dma_start(out=fs_sb[:], in_=fsml.ap())

            # ---- MLP input transposes: hbx halves -> xvt [128, 6, 64] ----
            xvt = cst.tile([128, KH, BC], dt.bfloat16)
            if do_mlp:
                nc.vector.memset(xvt[:], 0)
                hbe = hbx[0:BC, :, :].rearrange("b k e -> b (k e)")  # [64,320]
                hbo = hbx[64:64 + BC, :, :].rearrange("b k e -> b (k e)")
                for i, (src, idp) in enumerate(((hbe, 0), (hbo, 64))):
                    for kc in range(3):
                        kk = K1C[kc]
                        pt = psS.tile([128, BC], dt.bfloat16, tag="ps")
                        nc.tensor.transpose(
                            out=pt[:kk, :],
                            in_=src[:, kc * 128:kc * 128 + kk],
                            identity=wb_sb[idp:idp + 64, OFF_ID:OFF_ID + 64])
                        nc.vector.tensor_copy(out=xvt[:kk, 3 * i + kc, :],
                                              in_=pt[:kk, :])

            # ---- MLP (feature-major, fp8 weights x bf16 activations) ----
            layers = ((OFF_W1, K1C, 0), (OFF_W2, [128] * KH, 6),
                      (OFF_W3, [128] * KH, 12)) if do_mlp else ()
            cur_in = xvt
            ht = None
            for li, (woff, ksizes, boff) in enumerate(layers):
                ht = cst.tile([128, KH, BC], dt.bfloat16, tag=f"h{li + 1}t")
                nc.vector.memset(ht[:], 0)
                for mc in range(MT):
                    ms = _m_size(mc)
                    pm = psS.tile([128, BC], dt.float32, tag="ps")
                    for kc, kk in enumerate(ksizes):
                        nc.tensor.matmul(
                            out=pm[:ms, :],
                            lhsT=wq_sb[:kk, woff + kc * H + mc * 128:
                                       woff + kc * H + mc * 128 + ms],
                            rhs=cur_in[:kk, kc, :],
                            start=(kc == 0), stop=(kc == len(ksizes) - 1))
                    nc.scalar.activation(
                        out=ht[:ms, mc, :], in_=pm[:ms, :], func=relu,
                        bias=fs_sb[:ms, boff + mc:boff + mc + 1],
                        scale=1.0 / WS)
                cur_in = ht

            # ---- linear term (fp32 matmul): lrow = wlin^T @ xT + cnst ----
            lrow = psS.tile([1, BC], dt.float32, tag="ps")
            nc.tensor.matmul(out=lrow[:], lhsT=fs_sb[0:F, 19:20],
                             rhs=xt_sb[:], start=True, stop=True)
            osb = cst.tile([1, BC], dt.float32)
            nc.scalar.activation(out=osb[:], in_=lrow[:], func=copyf,
                                 bias=float(cnst_f))

            # ---- 4th MLP layer -> row [1, 64] ----
            ps4sb = None
            if do_mlp:
                ps4 = psS.tile([1, BC], dt.float32, tag="ps")
                for kc in range(KH):
                    nc.tensor.matmul(out=ps4[:],
                                     lhsT=wq_sb[:, OFF_W4 + kc:OFF_W4 + kc + 1],
                                     rhs=cur_in[:, kc, :],
                                     start=(kc == 0), stop=(kc == KH - 1))
                ps4sb = cst.tile([1, BC], dt.float32)
                nc.scalar.activation(out=ps4sb[:], in_=ps4[:], func=copyf,
                                     scale=1.0 / WS)

            # ---- FM operand staging: contiguous halves, all at base 0 ----
            # even half: strided geo slice -> contiguous (same base, DVE)
            # odd half: contiguous at base 64 (DVE), then partition-shift to
            # base 0 via SBUF->SBUF DMA (contiguous 2KB/partition descs)
            yvE = cst.tile([F, 512], dt.bfloat16)
            ypE = cst.tile([F, 512], dt.bfloat16)
            yvO = cst.tile([F, 512], dt.bfloat16)
            ypO = cst.tile([F, 512], dt.bfloat16)
            if do_fm2 or do_fm3:
                ytmp = cst.tile([128, 1024], dt.bfloat16)
                nc.vector.tensor_copy(out=yvE[:], in_=geo[0:F, :, 0:E])
                nc.vector.tensor_copy(out=ypE[:], in_=geo[0:F, :, E:2 * E])
                nc.vector.tensor_copy(out=ytmp[64:64 + F, 0:512],
                                      in_=geo[64:64 + F, :, 0:E])
                nc.vector.tensor_copy(out=ytmp[64:64 + F, 512:1024],
                                      in_=geo[64:64 + F, :, E:2 * E])
                nc.sync.dma_start(out=yvO[:], in_=ytmp[64:64 + F, 0:512])
                nc.sync.dma_start(out=ypO[:], in_=ytmp[64:64 + F, 512:1024])

            # ---- 2nd-order FM, per b-parity half ----
            fm2sb = None
            if do_fm2:
                r2 = cst.tile([F, BC], dt.float32)
                for h, yvh in ((0, yvE), (1, yvO)):
                    zps = psLG.tile([F, 512], dt.float32, tag="lg")
                    nc.tensor.matmul(out=zps[:],
                                     lhsT=wb_sb[0:F, OFF_AUP:OFF_AUP + F],
                                     rhs=yvh[:], start=True, stop=True)
                    p2 = strm.tile([F, 512], dt.float32, tag="p2")
                    nc.vector.tensor_tensor(out=p2[:], in0=yvh[:],
                                            in1=zps[:],
                                            op=mybir.AluOpType.mult)
                    nc.vector.tensor_reduce(
                        out=r2[:, h * 32:(h + 1) * 32],
                        in_=p2[:].rearrange("p (b e) -> p b e", e=E),
                        axis=mybir.AxisListType.X, op=mybir.AluOpType.add)
                fm2 = psS.tile([1, BC], dt.float32, tag="ps")
                nc.tensor.matmul(out=fm2[:], lhsT=fs_sb[0:F, 18:19],
                                 rhs=r2[:], start=True, stop=True)
                fm2sb = cst.tile([1, BC], dt.float32)
                nc.vector.tensor_copy(out=fm2sb[:], in_=fm2[:])

            # ---- 3rd-order FM, per b-parity half ----
            fm3sb = None
            if do_fm3:
                r3 = cst.tile([F, BC], dt.float32)
                for h, yph in ((0, ypE), (1, ypO)):
                    hr = psHR.tile([F, 512], dt.float32, tag=f"hr{h}")
                    for c in range(KH):
                        lps = psLG.tile([128, 512], dt.float32, tag="lg")
                        gps = psLG.tile([128, 512], dt.float32, tag="lg")
                        nc.tensor.matmul(out=lps[:],
                                         lhsT=wb_sb[0:F, OFF_SEL + c * 128:
                                                    OFF_SEL + (c + 1) * 128],
                                         rhs=yph[:], start=True, stop=True)
                        nc.tensor.matmul(out=gps[:],
                                         lhsT=wb_sb[0:F, OFF_W3M + c * 128:
                                                    OFF_W3M + (c + 1) * 128],
                                         rhs=yph[:], start=True, stop=True)
                        gsb = strm.tile([128, 512], dt.bfloat16, tag="gq")
                        nc.scalar.activation(out=gsb[:], in_=gps[:],
                                             func=copyf)
                        hsb = strm.tile([128, 512], dt.bfloat16, tag="hq")
                        nc.vector.tensor_tensor(out=hsb[:], in0=gsb[:],
                                                in1=lps[:],
                                                op=mybir.AluOpType.mult)
                        nc.tensor.matmul(
                            out=hr[:],
                            lhsT=wb_sb[:, OFF_SELR + c * F:
                                       OFF_SELR + (c + 1) * F],
                            rhs=hsb[:],
                            start=(c == 0), stop=(c == KH - 1))
                    f3 = strm.tile([F, 512], dt.float32, tag="p2")
                    nc.vector.tensor_tensor(out=f3[:], in0=yph[:],
                                            in1=hr[:],
                                            op=mybir.AluOpType.mult)
                    nc.vector.tensor_reduce(
                        out=r3[:, h * 32:(h + 1) * 32],
                        in_=f3[:].rearrange("p (b e) -> p b e", e=E),
                        axis=mybir.AxisListType.X, op=mybir.AluOpType.add)
                fm3 = psS.tile([1, BC], dt.float32, tag="ps")
                nc.tensor.matmul(out=fm3[:], lhsT=fs_sb[0:F, 18:19],
                                 rhs=r3[:], start=True, stop=True)
                fm3sb = cst.tile([1, BC], dt.float32)
                nc.vector.tensor_copy(out=fm3sb[:], in_=fm3[:])

            # ---- combine ----
            for term in (ps4sb, fm2sb, fm3sb):
                if term is not None:
                    nc.vector.tensor_tensor(out=osb[:], in0=osb[:],
                                            in1=term[:],
                                            op=mybir.AluOpType.add)
            nc.sync.dma_start(out=out_d.ap(), in_=osb[:])

    nc.compile()
    return nc


def _trip_index_map():
    m = {}
    for t, (i, j, k) in enumerate(combinations(range(F), 3)):
        m[(i, j, k)] = t
    return m


def _prep_consts(inputs_np):
    """Weight-derived constant blobs baked into the NEFF."""
    Ww = inputs_np["Ww"].astype(np.float64)
    bw = inputs_np["bw"].astype(np.float64)
    Wl = inputs_np["Wl"].astype(np.float64)
    bl = inputs_np["bl"].astype(np.float64)
    w_lin = (Ww.T @ Wl.T)[:, 0]  # [39]
    c_lin = float(bw @ Wl[0] + bl[0])

    edge_w = inputs_np["edge_w"].astype(np.float64)
    bn_g = inputs_np["bn_g"].astype(np.float64)
    bn_b = inputs_np["bn_b"].astype(np.float64)
    bn_m = inputs_np["bn_m"].astype(np.float64)
    bn_v = inputs_np["bn_v"].astype(np.float64)
    s = edge_w * bn_g / np.sqrt(bn_v + BN_EPS)
    c_fm = float(np.sum(edge_w * (bn_b - bn_m * bn_g / np.sqrt(bn_v + BN_EPS))))
    a_up = np.zeros((F, F), np.float64)
    for p, (i, j) in enumerate(combinations(range(F), 2)):
        a_up[i, j] = s[p]

    w3 = inputs_np["w3"].astype(np.float64)
    tmap = _trip_index_map()
    selL = np.zeros((F, PP), np.float64)
    selR = np.zeros((PP, F), np.float64)
    w3mat = np.zeros((F, PP), np.float64)
    for q, (i, j) in enumerate(PAIRS_JG):
        selL[i, q] = 1
        selR[q, j] = 1
        for k in range(j + 1, F):
            w3mat[k, q] = w3[tmap[(i, j, k)]]

    # wq: fp8 x16 weights; W1 K-space permuted to (even-f block, odd-f block)
    wq_blob = np.zeros((128, CQ), np.float64)
    W1 = inputs_np["W1"].astype(np.float64)  # [700, 624]
    w1p = np.zeros((H, 6 * 128), np.float64)
    for slot in range(20):          # even block: f = 2*slot
        w1p[:, slot * 16:(slot + 1) * 16] = \
            W1[:, (2 * slot) * 16:(2 * slot) * 16 + 16]
    for slot in range(19):          # odd block: f = 2*slot+1 (f=39 is zero)
        w1p[:, 384 + slot * 16:384 + (slot + 1) * 16] = \
            W1[:, (2 * slot + 1) * 16:(2 * slot + 1) * 16 + 16]
    # odd block lives at K-chunks 3..5 (rows 384..768 of the padded K space)
    for kc in range(6):
        kk = K1C[kc]
        k0 = [0, 128, 256, 384, 512, 640][kc]
        wq_blob[:kk, OFF_W1 + kc * H:OFF_W1 + kc * H + H] = \
            w1p[:, k0:k0 + kk].T

    def packK(dst_off, w):
        wt = w.T  # [K, M]
        for kc in range(KH):
            k0 = kc * 128
            kk = min(128, wt.shape[0] - k0)
            if kk > 0:
                wq_blob[:kk, dst_off + kc * H:dst_off + kc * H + wt.shape[1]] \
                    = wt[k0:k0 + kk]

    packK(OFF_W2, inputs_np["W2"].astype(np.float64))
    packK(OFF_W3, inputs_np["W3"].astype(np.float64))
    w4t = inputs_np["W4"].astype(np.float64).T  # [700, 1]
    for kc in range(KH):
        k0 = kc * 128
        kk = min(128, H - k0)
        if kk > 0:
            wq_blob[:kk, OFF_W4 + kc] = w4t[k0:k0 + kk, 0]
    wq_full = (wq_blob * WS).astype(FP8)

    # wb: bf16 matrices, pair matrices at both partition bases
    wb_blob = np.zeros((128, CB), np.float64)
    for lo in (0, 64):
        wb_blob[lo:lo + F, OFF_SEL:OFF_SEL + PP] = selL
        wb_blob[lo:lo + F, OFF_W3M:OFF_W3M + PP] = w3mat
        wb_blob[lo:lo + F, OFF_AUP:OFF_AUP + F] = a_up.T
        wb_blob[lo:lo + 64, OFF_ID:OFF_ID + 64] = np.eye(64)
    for c in range(KH):
        wb_blob[:, OFF_SELR + c * F:OFF_SELR + (c + 1) * F] = \
            selR[c * 128:(c + 1) * 128, :]
    wb_full = wb_blob.astype(BF16)

    # fsml: fp32 biases + ones + wlin
    fs_blob = np.zeros((128, CF), np.float32)
    for bi, nm in enumerate(("b1", "b2", "b3")):
        bv = inputs_np[nm].astype(np.float32)
        for mc in range(MT):
            m0 = mc * 128
            mm = min(128, H - m0)
            fs_blob[:mm, bi * 6 + mc] = bv[m0:m0 + mm]
    fs_blob[:, 18] = 1.0
    fs_blob[0:F, 19] = w_lin.astype(np.float32)

    cnst = float(c_lin + c_fm + float(inputs_np["b4"][0]))
    Evps16 = np.concatenate([inputs_np["Ev"].astype(BF16),
                             inputs_np["Eps"].astype(BF16)], axis=1)
    consts = {"evps": Evps16, "wq": wq_full, "wb": wb_full, "fsml": fs_blob}
    return consts, cnst


_CACHE = {}


def prepare(inputs):
    inputs_np = {k: np.asarray(v) for k, v in inputs.items()}
    key = (inputs_np["W1"].tobytes()[:256], inputs_np["Ev"].tobytes()[:256],
           os.environ.get("KSTAGE", "full"))
    if key not in _CACHE:
        consts, cnst = _prep_consts(inputs_np)
        _CACHE[key] = _build(consts, cnst)
    nc = _CACHE[key]

    ids_all = inputs_np["inputs"].astype(np.int64)  # [512, 39]
    in_maps = []
    for core in range(N_CORES):
        ids_c = ids_all[core * BC:(core + 1) * BC]  # [64, 39]
        idx_blob = np.zeros((128, NGEO + NHB + BC), np.int32)
        # geo calls: offsets[p, c] = ids[2c + p//64, p%64] for f=p%64<39
        p = np.arange(128)
        for c in range(NGEO):
            b = 2 * c + (p // 64)
            f = p % 64
            valid = f < F
            idx_blob[valid, c] = ids_c[b[valid], f[valid]]
        # hbx calls: offsets[p, k] = ids[BEO[p%64], 2k + p//64] (b-major MLP)
        for k in range(NHB):
            f = 2 * k + (p // 64)
            valid = f < F
            idx_blob[valid, NGEO + k] = \
                ids_c[BEO[p[valid] % 64], f[valid]]
        xt_blob = ids_c[BEO, :].T.astype(np.float32).copy()  # [39, 64]
        idx_blob[0:F, NGEO + NHB:] = xt_blob.view(np.int32)
        in_maps.append({"inp": idx_blob})
    return nc, in_maps


def kernel(**inputs) -> np.ndarray:
    nc, in_maps = prepare(inputs)
    if os.environ.get("KERNEL_BACKEND", "hw") == "sim":
        from concourse.bass_interp import CoreSim

        outs = []
        for c in range(N_CORES):
            sim = CoreSim(nc)
            for k, v in in_maps[c].items():
                sim.tensor(k)[:] = v
            sim.simulate()
            outs.append(sim.tensor("out").copy())
            if c == 0:
                print(f"[sim] core0 time: {sim.time:.0f} ns")
    else:
        res = run_bass_kernel_spmd(nc, in_maps, core_ids=list(range(N_CORES)))
        outs = [res.results[c]["out"] for c in range(N_CORES)]
    inv = np.argsort(BEO)
    return np.concatenate([o[0, inv] for o in outs]).astype(np.float32)


# revision 20
# speedup vs baseline: 1.2422x; 1.2422x over previous
"""AutoDeepFM forward on 8 Trainium2 NeuronCores (Bass/Tile), v3.

Key structure (data-parallel over batch, 64 rows/core):
  - Embedding table + all weights are baked into the NEFF as Const DRAM
    tensors (inline_tensor): loaded to HBM once at model-load, NOT bound
    per exec. Per-exec inputs are just the ids and the fp32 feature matrix
    (~50 KB), which collapses the per-exec dispatch overhead.
  - Gathers are chunked indirect DMAs ([128,1] offsets -- the only SWDGE
    ucode-supported shape), with flat orders chosen so the gather lands
    DIRECTLY in the compute layouts (no DRAM bounce):
      geo [128, 32, 32]: order n = b*64+f  -> even-b rows at partitions
        0..38, odd-b at 64..102, i.e. the field-major FM layout split in
        two partition groups (which then run matmuls CONCURRENTLY in
        disjoint PE row groups).
      hbx [128, 20, 16]: f-pair chunks -> batch-major MLP input split in
        two partition groups (f-even / f-odd), fixed up by PE transposes
        with a host-side permuted W1.
  - Batch order inside the kernel is even/odd interleaved; the host
    un-permutes after gathering results.
  - MLP in mixed fp8(weights, x16) x bf16(activations); scale undone via
    activation scale. 2nd/3rd order FM as in v2 but per b-parity half.
"""

import os
import functools
from itertools import combinations

import numpy as np
import ml_dtypes

import concourse.bass as bass
import concourse.mybir as mybir
import concourse.tile as tile
from concourse import bacc
from concourse.bass_utils import run_bass_kernel_spmd

BF16 = ml_dtypes.bfloat16
FP8 = ml_dtypes.float8_e4m3

B, F, E, V = 512, 39, 16, 1_000_000
N_CORES = 8
BC = B // N_CORES  # 64 batch rows per core
H = 700
PP = 768  # padded pair count (6 x 128)
KH = 6  # K chunks for hidden layers (700 -> 768)
MT = 6  # M tiles for hidden dims (700 -> 5x128+60)
BN_EPS = 1e-5
WS = 16.0  # fp8 weight scale

NGEO = 32  # geo gather calls (b-pair chunks)
NHB = 20   # hbx gather calls (f-pair chunks)

# W1 K-space: even-f block (20 f-slots x 16 = 320 rows, 3 chunks of
# 128/128/64) then odd-f block (20 slots x 16 = 320 rows incl. dummy f=39).
K1C = [128, 128, 64, 128, 128, 64]  # per-chunk K sizes, chunks 0-2 E, 3-5 O

# wq (fp8) column offsets: w1 6x700, w2 6x700, w3 6x700, w4 6
OFF_W1 = 0
OFF_W2 = 6 * H
OFF_W3 = 12 * H
OFF_W4 = 18 * H
CQ = 18 * H + KH

# wb (bf16) columns; sell/w3m/aupt live at partitions 0:39 AND 64:103
OFF_SEL = 0            # sell [39, 768] (both bases)
OFF_W3M = 768          # w3m [39, 768] (both bases)
OFF_SELR = 2 * 768     # selr [128, 6, 39]
OFF_AUP = OFF_SELR + 6 * F  # aupt [39, 39] (both bases)
OFF_ID = OFF_AUP + F   # ident [64, 64] (both bases)
CB = OFF_ID + 64

# fsml (fp32) columns: b1 [0:6], b2 [6:12], b3 [12:18], ones 18, wlin 19
CF = 20

PAIRS_JG = [(i, j) for j in range(1, F) for i in range(j)]

# even/odd batch permutation: kernel column j holds batch row BEO[j]
BEO = np.array([2 * j for j in range(BC // 2)] +
               [2 * j + 1 for j in range(BC // 2)], np.int64)


def _m_size(mc):
    return 128 if mc < MT - 1 else H - 128 * (MT - 1)  # 60 for the last tile


def _build(consts, cnst_f):
    """consts: dict name -> np.ndarray baked into the NEFF."""
    stage = os.environ.get("KSTAGE", "full")
    do_gather = stage in ("gather", "mlp", "fm2", "fm3", "full")
    do_mlp = stage in ("mlp", "fm2", "fm3", "full")
    do_fm2 = stage in ("fm2", "fm3", "full")
    do_fm3 = stage in ("fm3", "full")
    nc = bacc.Bacc("TRN2", target_bir_lowering=False, debug=False,
                   num_devices=N_CORES)
    dt = mybir.dt

    evps = nc.inline_tensor(consts["evps"], name="evps")
    wq = nc.inline_tensor(consts["wq"], name="wq")
    wb = nc.inline_tensor(consts["wb"], name="wb")
    fsml = nc.inline_tensor(consts["fsml"], name="fsml")

    idxs = nc.dram_tensor("idxs", [128, NGEO + NHB], dt.int32,
                          kind="ExternalInput")
    xt = nc.dram_tensor("xt", [F, BC], dt.float32, kind="ExternalInput")
    out_d = nc.dram_tensor("out", [1, BC], dt.float32, kind="ExternalOutput")

    relu = mybir.ActivationFunctionType.Relu
    copyf = mybir.ActivationFunctionType.Copy

    with tile.TileContext(nc) as tc:
        with (
            tc.tile_pool(name="cst", bufs=1) as cst,
            tc.tile_pool(name="stream", bufs=2) as strm,
            tc.tile_pool(name="ps_small", bufs=2, space="PSUM") as psS,
            tc.tile_pool(name="ps_hr", bufs=1, space="PSUM") as psHR,
            tc.tile_pool(name="ps_lg", bufs=4, space="PSUM") as psLG,
        ):
            # ---- per-exec input loads ----
            idx_sb = cst.tile([128, NGEO + NHB], dt.int32)
            nc.sync.dma_start(out=idx_sb[:], in_=idxs.ap())
            xt_sb = cst.tile([F, BC], dt.float32)
            nc.sync.dma_start(out=xt_sb[:], in_=xt.ap())

            # ---- chunked embedding gathers (one offset per partition) ----
            geo = cst.tile([128, NGEO, 2 * E], dt.bfloat16)
            hbx = cst.tile([128, NHB, E], dt.bfloat16)
            if do_gather:
                for c in range(NGEO):
                    nc.gpsimd.indirect_dma_start(
                        out=geo[:, c, :], out_offset=None, in_=evps.ap(),
                        in_offset=bass.IndirectOffsetOnAxis(
                            ap=idx_sb[:, c:c + 1], axis=0))
                for k in range(NHB):
                    nc.gpsimd.indirect_dma_start(
                        out=hbx[:, k, :], out_offset=None, in_=evps.ap(),
                        in_offset=bass.IndirectOffsetOnAxis(
                            ap=idx_sb[:, NGEO + k:NGEO + k + 1], axis=0))

            # ---- constant loads (overlap with gathers) ----
            wq_sb = cst.tile([128, CQ], dt.float8e4)
            if do_mlp:
                nc.sync.dma_start(out=wq_sb[:], in_=wq.ap())
            wb_sb = cst.tile([128, CB], dt.bfloat16)
            if do_mlp or do_fm2 or do_fm3:
                nc.sync.dma_start(out=wb_sb[:], in_=wb.ap())
            fs_sb = cst.tile([128, CF], dt.float32)
            nc.sync.dma_start(out=fs_sb[:], in_=fsml.ap())

            # ---- MLP input transposes: hbx halves -> xvt [128, 6, 64] ----
            xvt = cst.tile([128, KH, BC], dt.bfloat16)
            if do_mlp:
                nc.vector.memset(xvt[:], 0)
                hbe = hbx[0:BC, :, :].rearrange("b k e -> b (k e)")  # [64,320]
                hbo = hbx[64:64 + BC, :, :].rearrange("b k e -> b (k e)")
                for i, (src, idp) in enumerate(((hbe, 0), (hbo, 64))):
                    for kc in range(3):
                        kk = K1C[kc]
                        pt = psS.tile([128, BC], dt.bfloat16, tag="ps")
                        nc.tensor.transpose(
                            out=pt[:kk, :],
                            in_=src[:, kc * 128:kc * 128 + kk],
                            identity=wb_sb[idp:idp + 64, OFF_ID:OFF_ID + 64])
                        nc.vector.tensor_copy(out=xvt[:kk, 3 * i + kc, :],
                                              in_=pt[:kk, :])

            # ---- MLP (feature-major, fp8 weights x bf16 activations) ----
            layers = ((OFF_W1, K1C, 0), (OFF_W2, [128] * KH, 6),
                      (OFF_W3, [128] * KH, 12)) if do_mlp else ()
            cur_in = xvt
            ht = None
            for li, (woff, ksizes, boff) in enumerate(layers):
                ht = cst.tile([128, KH, BC], dt.bfloat16, tag=f"h{li + 1}t")
                nc.vector.memset(ht[:], 0)
                for mc in range(MT):
                    ms = _m_size(mc)
                    pm = psS.tile([128, BC], dt.float32, tag="ps")
                    for kc, kk in enumerate(ksizes):
                        nc.tensor.matmul(
                            out=pm[:ms, :],
                            lhsT=wq_sb[:kk, woff + kc * H + mc * 128:
                                       woff + kc * H + mc * 128 + ms],
                            rhs=cur_in[:kk, kc, :],
                            start=(kc == 0), stop=(kc == len(ksizes) - 1))
                    nc.scalar.activation(
                        out=ht[:ms, mc, :], in_=pm[:ms, :], func=relu,
                        bias=fs_sb[:ms, boff + mc:boff + mc + 1],
                        scale=1.0 / WS)
                cur_in = ht

            # ---- linear term (fp32 matmul): lrow = wlin^T @ xT + cnst ----
            lrow = psS.tile([1, BC], dt.float32, tag="ps")
            nc.tensor.matmul(out=lrow[:], lhsT=fs_sb[0:F, 19:20],
                             rhs=xt_sb[:], start=True, stop=True)
            osb = cst.tile([1, BC], dt.float32)
            nc.scalar.activation(out=osb[:], in_=lrow[:], func=copyf,
                                 bias=float(cnst_f))

            # ---- 4th MLP layer -> row [1, 64] ----
            ps4sb = None
            if do_mlp:
                ps4 = psS.tile([1, BC], dt.float32, tag="ps")
                for kc in range(KH):
                    nc.tensor.matmul(out=ps4[:],
                                     lhsT=wq_sb[:, OFF_W4 + kc:OFF_W4 + kc + 1],
                                     rhs=cur_in[:, kc, :],
                                     start=(kc == 0), stop=(kc == KH - 1))
                ps4sb = cst.tile([1, BC], dt.float32)
                nc.scalar.activation(out=ps4sb[:], in_=ps4[:], func=copyf,
                                     scale=1.0 / WS)

            # ---- FM operand staging: contiguous halves, all at base 0 ----
            # even half: strided geo slice -> contiguous (same base, DVE)
            # odd half: contiguous at base 64 (DVE), then partition-shift to
            # base 0 via SBUF->SBUF DMA (contiguous 2KB/partition descs)
            yvE = cst.tile([F, 512], dt.bfloat16)
            ypE = cst.tile([F, 512], dt.bfloat16)
            yvO = cst.tile([F, 512], dt.bfloat16)
            ypO = cst.tile([F, 512], dt.bfloat16)
            if do_fm2 or do_fm3:
                ytmp = cst.tile([128, 1024], dt.bfloat16)
                nc.vector.tensor_copy(out=yvE[:], in_=geo[0:F, :, 0:E])
                nc.vector.tensor_copy(out=ypE[:], in_=geo[0:F, :, E:2 * E])
                nc.vector.tensor_copy(out=ytmp[64:64 + F, 0:512],
                                      in_=geo[64:64 + F, :, 0:E])
                nc.vector.tensor_copy(out=ytmp[64:64 + F, 512:1024],
                                      in_=geo[64:64 + F, :, E:2 * E])
                nc.sync.dma_start(out=yvO[:], in_=ytmp[64:64 + F, 0:512])
                nc.sync.dma_start(out=ypO[:], in_=ytmp[64:64 + F, 512:1024])

            # ---- 2nd-order FM, per b-parity half ----
            fm2sb = None
            if do_fm2:
                r2 = cst.tile([F, BC], dt.float32)
                for h, yvh in ((0, yvE), (1, yvO)):
                    zps = psLG.tile([F, 512], dt.float32, tag="lg")
                    nc.tensor.matmul(out=zps[:],
                                     lhsT=wb_sb[0:F, OFF_AUP:OFF_AUP + F],
                                     rhs=yvh[:], start=True, stop=True)
                    p2 = strm.tile([F, 512], dt.float32, tag="p2")
                    nc.vector.tensor_tensor(out=p2[:], in0=yvh[:],
                                            in1=zps[:],
                                            op=mybir.AluOpType.mult)
                    nc.vector.tensor_reduce(
                        out=r2[:, h * 32:(h + 1) * 32],
                        in_=p2[:].rearrange("p (b e) -> p b e", e=E),
                        axis=mybir.AxisListType.X, op=mybir.AluOpType.add)
                fm2 = psS.tile([1, BC], dt.float32, tag="ps")
                nc.tensor.matmul(out=fm2[:], lhsT=fs_sb[0:F, 18:19],
                                 rhs=r2[:], start=True, stop=True)
                fm2sb = cst.tile([1, BC], dt.float32)
                nc.vector.tensor_copy(out=fm2sb[:], in_=fm2[:])

            # ---- 3rd-order FM, per b-parity half ----
            fm3sb = None
            if do_fm3:
                r3 = cst.tile([F, BC], dt.float32)
                for h, yph in ((0, ypE), (1, ypO)):
                    hr = psHR.tile([F, 512], dt.float32, tag=f"hr{h}")
                    for c in range(KH):
                        lps = psLG.tile([128, 512], dt.float32, tag="lg")
                        gps = psLG.tile([128, 512], dt.float32, tag="lg")
                        nc.tensor.matmul(out=lps[:],
                                         lhsT=wb_sb[0:F, OFF_SEL + c * 128:
                                                    OFF_SEL + (c + 1) * 128],
                                         rhs=yph[:], start=True, stop=True)
                        nc.tensor.matmul(out=gps[:],
                                         lhsT=wb_sb[0:F, OFF_W3M + c * 128:
                                                    OFF_W3M + (c + 1) * 128],
                                         rhs=yph[:], start=True, stop=True)
                        gsb = strm.tile([128, 512], dt.bfloat16, tag="gq")
                        nc.scalar.activation(out=gsb[:], in_=gps[:],
                                             func=copyf)
                        hsb = strm.tile([128, 512], dt.bfloat16, tag="hq")
                        nc.vector.tensor_tensor(out=hsb[:], in0=gsb[:],
                                                in1=lps[:],
                                                op=mybir.AluOpType.mult)
                        nc.tensor.matmul(
                            out=hr[:],
                            lhsT=wb_sb[:, OFF_SELR + c * F:
                                       OFF_SELR + (c + 1) * F],
                            rhs=hsb[:],
                            start=(c == 0), stop=(c == KH - 1))
                    f3 = strm.tile([F, 512], dt.float32, tag="p2")
                    nc.vector.tensor_tensor(out=f3[:], in0=yph[:],
                                            in1=hr[:],
                                            op=mybir.AluOpType.mult)
                    nc.vector.tensor_reduce(
                        out=r3[:, h * 32:(h + 1) * 32],
                        in_=f3[:].rearrange("p (b e) -> p b e", e=E),
                        axis=mybir.AxisListType.X, op=mybir.AluOpType.add)
                fm3 = psS.tile([1, BC], dt.float32, tag="ps")
                nc.tensor.matmul(out=fm3[:], lhsT=fs_sb[0:F, 18:19],
                                 rhs=r3[:], start=True, stop=True)
                fm3sb = cst.tile([1, BC], dt.float32)
                nc.vector.tensor_copy(out=fm3sb[:], in_=fm3[:])

            # ---- combine ----
            for term in (ps4sb, fm2sb, fm3sb):
                if term is not None:
                    nc.vector.tensor_tensor(out=osb[:], in0=osb[:],
                                            in1=term[:],
                                            op=mybir.AluOpType.add)
            nc.sync.dma_start(out=out_d.ap(), in_=osb[:])

    nc.compile()
    return nc


def _trip_index_map():
    m = {}
    for t, (i, j, k) in enumerate(combinations(range(F), 3)):
        m[(i, j, k)] = t
    return m


def _prep_consts(inputs_np):
    """Weight-derived constant blobs baked into the NEFF."""
    Ww = inputs_np["Ww"].astype(np.float64)
    bw = inputs_np["bw"].astype(np.float64)
    Wl = inputs_np["Wl"].astype(np.float64)
    bl = inputs_np["bl"].astype(np.float64)
    w_lin = (Ww.T @ Wl.T)[:, 0]  # [39]
    c_lin = float(bw @ Wl[0] + bl[0])

    edge_w = inputs_np["edge_w"].astype(np.float64)
    bn_g = inputs_np["bn_g"].astype(np.float64)
    bn_b = inputs_np["bn_b"].astype(np.float64)
    bn_m = inputs_np["bn_m"].astype(np.float64)
    bn_v = inputs_np["bn_v"].astype(np.float64)
    s = edge_w * bn_g / np.sqrt(bn_v + BN_EPS)
    c_fm = float(np.sum(edge_w * (bn_b - bn_m * bn_g / np.sqrt(bn_v + BN_EPS))))
    a_up = np.zeros((F, F), np.float64)
    for p, (i, j) in enumerate(combinations(range(F), 2)):
        a_up[i, j] = s[p]

    w3 = inputs_np["w3"].astype(np.float64)
    tmap = _trip_index_map()
    selL = np.zeros((F, PP), np.float64)
    selR = np.zeros((PP, F), np.float64)
    w3mat = np.zeros((F, PP), np.float64)
    for q, (i, j) in enumerate(PAIRS_JG):
        selL[i, q] = 1
        selR[q, j] = 1
        for k in range(j + 1, F):
            w3mat[k, q] = w3[tmap[(i, j, k)]]

    # wq: fp8 x16 weights; W1 K-space permuted to (even-f block, odd-f block)
    wq_blob = np.zeros((128, CQ), np.float64)
    W1 = inputs_np["W1"].astype(np.float64)  # [700, 624]
    w1p = np.zeros((H, 6 * 128), np.float64)
    for slot in range(20):          # even block: f = 2*slot
        w1p[:, slot * 16:(slot + 1) * 16] = \
            W1[:, (2 * slot) * 16:(2 * slot) * 16 + 16]
    for slot in range(19):          # odd block: f = 2*slot+1 (f=39 is zero)
        w1p[:, 384 + slot * 16:384 + (slot + 1) * 16] = \
            W1[:, (2 * slot + 1) * 16:(2 * slot + 1) * 16 + 16]
    # odd block lives at K-chunks 3..5 (rows 384..768 of the padded K space)
    for kc in range(6):
        kk = K1C[kc]
        k0 = [0, 128, 256, 384, 512, 640][kc]
        wq_blob[:kk, OFF_W1 + kc * H:OFF_W1 + kc * H + H] = \
            w1p[:, k0:k0 + kk].T

    def packK(dst_off, w):
        wt = w.T  # [K, M]
        for kc in range(KH):
            k0 = kc * 128
            kk = min(128, wt.shape[0] - k0)
            if kk > 0:
                wq_blob[:kk, dst_off + kc * H:dst_off + kc * H + wt.shape[1]] \
                    = wt[k0:k0 + kk]

    packK(OFF_W2, inputs_np["W2"].astype(np.float64))
    packK(OFF_W3, inputs_np["W3"].astype(np.float64))
    w4t = inputs_np["W4"].astype(np.float64).T  # [700, 1]
    for kc in range(KH):
        k0 = kc * 128
        kk = min(128, H - k0)
        if kk > 0:
            wq_blob[:kk, OFF_W4 + kc] = w4t[k0:k0 + kk, 0]
    wq_full = (wq_blob * WS).astype(FP8)

    # wb: bf16 matrices, pair matrices at both partition bases
    wb_blob = np.zeros((128, CB), np.float64)
    for lo in (0, 64):
        wb_blob[lo:lo + F, OFF_SEL:OFF_SEL + PP] = selL
        wb_blob[lo:lo + F, OFF_W3M:OFF_W3M + PP] = w3mat
        wb_blob[lo:lo + F, OFF_AUP:OFF_AUP + F] = a_up.T
        wb_blob[lo:lo + 64, OFF_ID:OFF_ID + 64] = np.eye(64)
    for c in range(KH):
        wb_blob[:, OFF_SELR + c * F:OFF_SELR + (c + 1) * F] = \
            selR[c * 128:(c + 1) * 128, :]
    wb_full = wb_blob.astype(BF16)

    # fsml: fp32 biases + ones + wlin
    fs_blob = np.zeros((128, CF), np.float32)
    for bi, nm in enumerate(("b1", "b2", "b3")):
        bv = inputs_np[nm].astype(np.float32)
        for mc in range(MT):
            m0 = mc * 128
            mm = min(128, H - m0)
            fs_blob[:mm, bi * 6 + mc] = bv[m0:m0 + mm]
    fs_blob[:, 18] = 1.0
    fs_blob[0:F, 19] = w_lin.astype(np.float32)

    cnst = float(c_lin + c_fm + float(inputs_np["b4"][0]))
    Evps16 = np.concatenate([inputs_np["Ev"].astype(BF16),
                             inputs_np["Eps"].astype(BF16)], axis=1)
    consts = {"evps": Evps16, "wq": wq_full, "wb": wb_full, "fsml": fs_blob}
    return consts, cnst


_CACHE = {}


def prepare(inputs):
    inputs_np = {k: np.asarray(v) for k, v in inputs.items()}
    key = (inputs_np["W1"].tobytes()[:256], inputs_np["Ev"].tobytes()[:256],
           os.environ.get("KSTAGE", "full"))
    if key not in _CACHE:
        consts, cnst = _prep_consts(inputs_np)
        _CACHE[key] = _build(consts, cnst)
    nc = _CACHE[key]

    ids_all = inputs_np["inputs"].astype(np.int64)  # [512, 39]
    in_maps = []
    for core in range(N_CORES):
        ids_c = ids_all[core * BC:(core + 1) * BC]  # [64, 39]
        idx_blob = np.zeros((128, NGEO + NHB), np.int32)
        # geo calls: offsets[p, c] = ids[2c + p//64, p%64] for f=p%64<39
        p = np.arange(128)
        for c in range(NGEO):
            b = 2 * c + (p // 64)
            f = p % 64
            valid = f < F
            idx_blob[valid, c] = ids_c[b[valid], f[valid]]
        # hbx calls: offsets[p, k] = ids[BEO[p%64], 2k + p//64] (b-major MLP)
        for k in range(NHB):
            f = 2 * k + (p // 64)
            valid = f < F
            idx_blob[valid, NGEO + k] = \
                ids_c[BEO[p[valid] % 64], f[valid]]
        xt_blob = ids_c[BEO, :].T.astype(np.float32).copy()  # [39, 64]
        in_maps.append({"idxs": idx_blob, "xt": xt_blob})
    return nc, in_maps


def kernel(**inputs) -> np.ndarray:
    nc, in_maps = prepare(inputs)
    if os.environ.get("KERNEL_BACKEND", "hw") == "sim":
        from concourse.bass_interp import CoreSim

        outs = []
        for c in range(N_CORES):
            sim = CoreSim(nc)
            for k, v in in_maps[c].items():
                sim.tensor(k)[:] = v
            sim.simulate()
            outs.append(sim.tensor("out").copy())
            if c == 0:
                print(f"[sim] core0 time: {sim.time:.0f} ns")
    else:
        res = run_bass_kernel_spmd(nc, in_maps, core_ids=list(range(N_CORES)))
        outs = [res.results[c]["out"] for c in range(N_CORES)]
    inv = np.argsort(BEO)
    return np.concatenate([o[0, inv] for o in outs]).astype(np.float32)
